# revision 1
# baseline (speedup 1.0000x reference)
"""LIF neuron scan kernel for Trainium2 (8 NeuronCores, raw Bass SPMD).

Math (per timestep, fp32): v = v_prev*0.5 + x + r; s = (v > 0); v *= (1-s).
Reset+leak fold to v = 0.5*min(v_prev, 0) + (x + r).  With the exact fp32
rescaling w_t = 2^t * v_t (power-of-two scaling commutes with IEEE rounding)
the recurrence becomes a single fused op per step:
    w_t = min(w_{t-1}, 0) + U'_t,   U'_t = 2^t * (x_t + r_t)
and s_t = (w_t > 0) = (v_t > 0).  2^t*x, 2^t*r are prescaled on host (exact;
max |w| ~ 2^99 * 16 << fp32 max).  Spikes are emitted as uint8 via the
activation engine's Sign (u8 saturating cast maps {-1,0,1}->{0,0,1}); host
decodes (raw == 1) -> f32.

Sharding: data-parallel along batch; core i gets inp[:, 8i:8i+8, :].
Per-core DRAM layout is pre-transposed on host to [128 partitions, T, 128]
so every DMA line is contiguous per partition.  Warm-up chunks (4, 6 steps)
collapse the pipeline ramp before the steady 10-step chunks.  The SP ring
carries ONLY input DMAs, issued eagerly 4 chunks ahead (gated solely on
buffer release) so the DMA engines stream continuously; Act carries the
spike out-DMAs.

Write-visibility discipline (observed on HW): an engine's posted SBUF writes
can lag its semaphore increment by over a microsecond under DMA pressure, so
a consumer on another engine (or a DMA read) may see stale data.  Publishes
therefore carry a one-producer-step lag (Act reads chunk c's w after DVE
retires chunk c+1; the out-DMA reads chunk c's spikes after Sign c+1), with
single trailing drain-incs covering the last chunks.  GpSimd ops are
framework-auto-drained before their increments.
"""
import sys
sys.path.insert(0, "/opt/trn_rl_repo")
import numpy as np
import concourse.bass as bass
from concourse import mybir
from concourse.bass_utils import run_bass_kernel_spmd

F32 = mybir.dt.float32
U8 = mybir.dt.uint8
T, B, N = 100, 64, 2048
NCORES = 8
B_LOC = B // NCORES
P = 128
F = (B_LOC * N) // P      # 128 free elems per step
SIZES = [4, 6] + [10] * 8 + [6, 4]  # warm-up / steady / cool-down, sums to T
OFFS = [sum(SIZES[:i]) for i in range(len(SIZES))]
C = len(SIZES)
KF = max(SIZES) * F


def _build_nc():
    nc = bass.Bass()
    x_ext = nc.dram_tensor("x", [P, T * F], F32, kind="ExternalInput")
    r_ext = nc.dram_tensor("r", [P, T * F], F32, kind="ExternalInput")
    s_ext = nc.dram_tensor("s", [P, T * F], U8, kind="ExternalOutput")

    with (
        nc.sbuf_tensor([P, T * F], F32) as xb,
        nc.sbuf_tensor([P, T * F], F32) as rb,
        nc.sbuf_tensor([P, 5, KF], F32, side="right") as ub,
        nc.sbuf_tensor([P, 5, KF], F32, side="right") as wb,
        nc.sbuf_tensor([P, T * F], U8) as sb,
        nc.sbuf_tensor([P, F], F32, side="right") as z0,
        nc.semaphore() as sem_x,
        nc.semaphore() as sem_r,
        nc.semaphore() as pool_done,
        nc.semaphore() as dve_done,
        nc.semaphore() as act_done,
        nc.semaphore() as sem_out,
        nc.Block() as block,
    ):
        @block.sync
        def _(sync):
            # full inputs fit in SBUF: issue everything up front, zero reuse
            # gating -- the ring streams continuously at its burst rate.
            for c in range(C):
                lo, kf = OFFS[c] * F, SIZES[c] * F
                sync.dma_start(xb[:, lo:lo+kf], x_ext[:, lo:lo+kf]).then_inc(sem_x, 16)
                sync.dma_start(rb[:, lo:lo+kf], r_ext[:, lo:lo+kf]).then_inc(sem_r, 16)

        @block.gpsimd
        def _(pool):
            # chunks 0-1 are added by DVE itself (shorter ramp)
            for c in range(2, C):
                lo, kf = OFFS[c] * F, SIZES[c] * F
                pool.wait_ge(sem_x, 16 * (c + 1))
                pool.wait_ge(sem_r, 16 * (c + 1))
                if c >= 5:
                    pool.wait_ge(dve_done, c - 4)    # ub[c%5] freed by DVE(c-5)
                nc.gpsimd.tensor_tensor(ub[:, c % 5, :kf], xb[:, lo:lo+kf],
                                        rb[:, lo:lo+kf],
                                        mybir.AluOpType.add).then_inc(pool_done, 1)

        @block.vector
        def _(vector):
            nc.vector.memset(z0[:], 0.0)
            for c in range(C):
                w = c % 5
                kf = SIZES[c] * F
                if c >= 5:
                    vector.wait_ge(act_done, c - 4)  # wb[w] freed by Act(c-5)
                if c < 2:
                    # DVE adds its own warm-up chunks: no Pool hop at ramp.
                    lo = OFFS[c] * F
                    vector.wait_ge(sem_x, 16 * (c + 1))
                    nc.vector.tensor_tensor(
                        ub[:, c % 5, :kf], xb[:, lo:lo+kf], rb[:, lo:lo+kf],
                        mybir.AluOpType.add
                    ).wait_op(sem_r, 16 * (c + 1), "sem-ge").then_inc(pool_done, 1)
                for k in range(SIZES[c]):
                    if c == 0 and k == 0:
                        wprev = z0[:]
                    elif k == 0:
                        wprev = wb[:, (c-1) % 5, (SIZES[c-1]-1)*F:SIZES[c-1]*F]
                    else:
                        wprev = wb[:, w, (k-1)*F:k*F]
                    ins = nc.vector.scalar_tensor_tensor(
                        wb[:, w, k*F:(k+1)*F], wprev, 0.0,
                        ub[:, c % 5, k*F:(k+1)*F],
                        mybir.AluOpType.min, mybir.AluOpType.add)
                    if k == 0 and c >= 2:
                        ins.wait_op(pool_done, c + 1, "sem-ge")  # fused wait
                    if k == SIZES[c] - 1:
                        ins.then_inc(dve_done, 1)
            # final publish: flush the last chunk's posted writes
            vector.maybe_drain_then_inc((dve_done, 1))

        @block.scalar
        def _(act):
            for c in range(C):
                kf = SIZES[c] * F
                lo = OFFS[c] * F
                # one-chunk lag: chunk c's wb writes settle while DVE runs
                # chunk c+1 (dve_done saturates at C+1 via the final
                # drain-inc, which also covers the last chunk; DMA traffic is
                # gone by the cool-down, so settle there is fast).
                act.wait_ge(dve_done, min(c + 2, C + 1))
                nc.scalar.activation(sb[:, lo:lo+kf], wb[:, c % 5, :kf],
                                     mybir.ActivationFunctionType.Sign).then_inc(act_done, 1)
                if c >= 1:
                    # out(c-1): its sb writes settled during sign(c)
                    o = c - 1
                    olo, okf = OFFS[o] * F, SIZES[o] * F
                    act.dma_start(s_ext[:, olo:olo+okf], sb[:, olo:olo+okf]).then_inc(sem_out, 16)
            # flush the last sign's posted writes, then emit its chunk
            act.drain()
            olo, okf = OFFS[C-1] * F, SIZES[C-1] * F
            act.dma_start(s_ext[:, olo:olo+okf], sb[:, olo:olo+okf]).then_inc(sem_out, 16)

    return nc


_SCALE = np.exp2(np.arange(T, dtype=np.float32)).astype(np.float32)


def _shard(inp: np.ndarray, rec: np.ndarray) -> list[dict[str, np.ndarray]]:
    # prescale by 2^t (exact in fp32), then per-core transpose to [P, T*F]
    xs_all = inp.reshape(T, B, N) * _SCALE[:, None, None]
    rs_all = rec.reshape(T, B, N) * _SCALE[:, None, None]
    in_maps = []
    for i in range(NCORES):
        xs = xs_all[:, i*B_LOC:(i+1)*B_LOC, :].reshape(T, P, F)
        rs = rs_all[:, i*B_LOC:(i+1)*B_LOC, :].reshape(T, P, F)
        in_maps.append({
            "x": np.ascontiguousarray(xs.transpose(1, 0, 2)).reshape(P, T * F),
            "r": np.ascontiguousarray(rs.transpose(1, 0, 2)).reshape(P, T * F),
        })
    return in_maps


def kernel(inp: np.ndarray, rec: np.ndarray) -> np.ndarray:
    inp = np.asarray(inp, dtype=np.float32)
    rec = np.asarray(rec, dtype=np.float32)
    nc = _build_nc()
    in_maps = _shard(inp, rec)
    res = run_bass_kernel_spmd(nc, in_maps, list(range(NCORES)))
    outs = []
    for i in range(NCORES):
        raw = res.results[i]["s"].reshape(P, T, F)          # uint8
        s = (raw == 1).astype(np.float32).transpose(1, 0, 2)  # [T, P, F]
        outs.append(s.reshape(T, B_LOC, N))
    return np.concatenate(outs, axis=1)

